# revision 1
# baseline (speedup 1.0000x reference)
"""Trainium2 Bass kernel for nn_MixtralSparseMoeBlock (moe_routing).

Strategy (FFN-dimension sharding, "tensor parallel over F"):
  - Routing (softmax + top-2) is computed host-side with jax-on-CPU,
    replicating the reference bit-for-bit; router_logits are ALSO computed
    on-device (fp32 matmul) and returned as the second output.
  - The (token, expert) pairs are sorted by expert, padded per-expert to a
    multiple of 128.  Every core processes ALL pairs but only a 512-wide
    slice of the FFN dimension F=4096 (F/8 per core).  SwiGLU is elementwise
    in F, so partial outputs  (silu(x@W1[:, s]) * (x@W3[:, s])) @ W2[s, :]
    sum exactly to the full expert output.  All 8 cores therefore run an
    IDENTICAL instruction stream on different weight slices: perfect load
    balance, and each weight element is read from HBM exactly once per core.
  - Per-pair combine weights (renormalized top-2 softmax) are applied on
    device during the PSUM->SBUF copyback; the host sums the 8 partial
    outputs and scatter-adds them into the final [B,S,D] output.

Compute is done in bf16 (PE runs 1 cycle/row for bf16 vs 4 for fp32) with
fp32 PSUM accumulation.  Per-core work ~52 GFLOP -> ~670 us at 78.6 TF/s.
"""

import os
import sys
from contextlib import ExitStack

import numpy as np

sys.path.insert(0, "/opt/trn_rl_repo")

import ml_dtypes

BF16 = np.dtype(ml_dtypes.bfloat16)

# Problem shapes (hardcoded per harness contract)
B, S, D, E, F = 4, 2048, 1024, 8, 4096
T = B * S                      # 8192 tokens
TOPK = 2
NCORES = 8
FSL = F // NCORES              # 512: F-slice per core
P = 128                        # partitions
DC = D // P                    # 8 contraction chunks over D
FC = FSL // P                  # 4 contraction chunks over F-slice
TR = T // NCORES               # 1024 router tokens per core
TOKBLK = 512                   # token block (matmul moving dim)

# results of the last device run, for test.py introspection
LAST_RESULTS = None
LAST_NC = None


def _route_host(xt, gate_w):
    """Replicate the reference routing exactly, on the CPU jax backend."""
    import jax
    import jax.numpy as jnp

    cpu = jax.devices("cpu")[0]

    def fn(xt, gate_w):
        router_logits = xt @ gate_w.T
        probs = jax.nn.softmax(router_logits.astype(jnp.float32), axis=-1)
        top_w, top_idx = jax.lax.top_k(probs, TOPK)
        top_w = top_w / jnp.sum(top_w, axis=-1, keepdims=True)
        return router_logits, top_w, top_idx

    with jax.default_device(cpu):
        logits, top_w, top_idx = jax.jit(fn)(xt, gate_w)
    return (np.asarray(logits), np.asarray(top_w), np.asarray(top_idx))


def _build_schedule(top_idx, top_w):
    """Sort (token, expert) pairs by expert, pad each expert segment to a
    multiple of 128.  Returns token gather list, per-pair weights, per-expert
    (offset, real_count) and the static block schedule."""
    tok_list = []
    w_list = []
    seg = []           # (expert, offset, real_count)
    blocks = []        # (expert, t0, nb) with nb a multiple of 128, <= 512
    off = 0
    for e in range(E):
        rows, cols = np.nonzero(top_idx == e)
        c = len(rows)
        if c == 0:
            continue
        cp = ((c + P - 1) // P) * P
        tok = np.zeros(cp, dtype=np.int64)
        tok[:c] = rows
        w = np.zeros(cp, dtype=np.float32)
        w[:c] = top_w[rows, cols]
        tok_list.append(tok)
        w_list.append(w)
        seg.append((e, off, c))
        t0 = off
        rem = cp
        while rem > 0:
            nb = min(TOKBLK, rem)
            blocks.append((e, t0, nb))
            t0 += nb
            rem -= nb
        off += cp
    pair_tok = np.concatenate(tok_list)
    pair_w = np.concatenate(w_list)
    return pair_tok, pair_w, seg, blocks, off  # off == T2


def _build_program(blocks, T2):
    import concourse.bacc as bacc
    import concourse.mybir as mybir
    import concourse.tile as tile

    bf = mybir.dt.bfloat16
    f32 = mybir.dt.float32
    AF = mybir.ActivationFunctionType

    nc = bacc.Bacc("TRN2", target_bir_lowering=False, debug=False,
                   num_devices=NCORES)

    XG = nc.dram_tensor("xg", [P, DC, T2], bf, kind="ExternalInput")
    WG = nc.dram_tensor("wg", [P, T2 // P], f32, kind="ExternalInput")
    W1S = nc.dram_tensor("w1s", [E, P, DC, FSL], bf, kind="ExternalInput")
    W3S = nc.dram_tensor("w3s", [E, P, DC, FSL], bf, kind="ExternalInput")
    W2S = nc.dram_tensor("w2s", [E, P, FC, D], bf, kind="ExternalInput")
    XR = nc.dram_tensor("xr", [P, DC, TR], f32, kind="ExternalInput")
    GWT = nc.dram_tensor("gwt", [P, DC, E], f32, kind="ExternalInput")
    YP = nc.dram_tensor("yp", [T2, D], bf, kind="ExternalOutput")
    LG = nc.dram_tensor("lg", [TR, E], f32, kind="ExternalOutput")

    with tile.TileContext(nc) as tc, ExitStack() as ctx:
        const = ctx.enter_context(tc.tile_pool(name="const", bufs=1))
        xpool = ctx.enter_context(tc.tile_pool(name="xpool", bufs=3))
        wpool = ctx.enter_context(tc.tile_pool(name="wpool", bufs=2))
        hpool = ctx.enter_context(tc.tile_pool(name="hpool", bufs=3))
        spool = ctx.enter_context(tc.tile_pool(name="spool", bufs=3))
        ypool = ctx.enter_context(tc.tile_pool(name="ypool", bufs=4))
        ps1 = ctx.enter_context(tc.tile_pool(name="ps1", bufs=2, space="PSUM"))
        ps2 = ctx.enter_context(tc.tile_pool(name="ps2", bufs=2, space="PSUM"))
        psr = ctx.enter_context(tc.tile_pool(name="psr", bufs=1, space="PSUM"))

        # ---------------- router: logits = x @ gate_w.T (fp32) -------------
        gwt = const.tile([P, DC, E], f32)
        nc.sync.dma_start(gwt[:], GWT[:])
        xr = const.tile([P, DC, TR], f32)
        nc.sync.dma_start(xr[:], XR[:])
        lg_sb = const.tile([P, TR // P, E], f32)
        for tt in range(TR // P):
            pl = psr.tile([P, E], f32, tag="pl", name=f"pl{tt}")
            for dc in range(DC):
                nc.tensor.matmul(pl[:], lhsT=xr[:, dc, tt * P:(tt + 1) * P],
                                 rhs=gwt[:, dc, :],
                                 start=(dc == 0), stop=(dc == DC - 1))
            nc.scalar.copy(lg_sb[:, tt, :], pl[:])
        nc.sync.dma_start(LG.rearrange("(tt p) e -> p tt e", p=P), lg_sb[:])

        # per-pair combine weights
        wgt = const.tile([P, T2 // P], f32)
        nc.sync.dma_start(wgt[:], WG[:])

        # ---------------- MoE main ----------------------------------------
        weights = {}

        def load_weights(e):
            w1t = wpool.tile([P, DC, FSL], bf, tag="w1", name=f"w1_{e}")
            nc.sync.dma_start(w1t[:], W1S[e])
            w3t = wpool.tile([P, DC, FSL], bf, tag="w3", name=f"w3_{e}")
            nc.sync.dma_start(w3t[:], W3S[e])
            w2t = wpool.tile([P, FC, D], bf, tag="w2", name=f"w2_{e}")
            nc.sync.dma_start(w2t[:], W2S[e])
            return w1t, w3t, w2t

        def stage1(blk):
            """uT = (x@W1slice).T, vT = (x@W3slice).T, hT = silu(uT)*vT"""
            e, t0, nb = blk
            w1t, w3t, _ = weights[e]
            xgt = xpool.tile([P, DC, TOKBLK], bf, tag="xg", name=f"xg_{t0}")
            nc.sync.dma_start(xgt[:, :, :nb], XG[:, :, t0:t0 + nb])
            ht = hpool.tile([P, FC, TOKBLK], bf, tag="ht", name=f"ht_{t0}")
            for fc in range(FC):
                pu = ps1.tile([P, TOKBLK], f32, tag="pu", name=f"pu_{t0}_{fc}")
                pv = ps1.tile([P, TOKBLK], f32, tag="pv", name=f"pv_{t0}_{fc}")
                for dc in range(DC):
                    nc.tensor.matmul(pu[:, :nb],
                                     lhsT=w1t[:, dc, fc * P:(fc + 1) * P],
                                     rhs=xgt[:, dc, :nb],
                                     start=(dc == 0), stop=(dc == DC - 1))
                for dc in range(DC):
                    nc.tensor.matmul(pv[:, :nb],
                                     lhsT=w3t[:, dc, fc * P:(fc + 1) * P],
                                     rhs=xgt[:, dc, :nb],
                                     start=(dc == 0), stop=(dc == DC - 1))
                su = spool.tile([P, TOKBLK], bf, tag="su", name=f"su_{t0}_{fc}")
                nc.scalar.activation(su[:, :nb], pu[:, :nb], AF.Silu)
                nc.vector.tensor_mul(ht[:, fc, :nb], su[:, :nb], pv[:, :nb])
            return ht

        def stage2(blk, ht):
            """y = (hT.T @ W2slice) * w_combine, streamed to DRAM"""
            e, t0, nb = blk
            _, _, w2t = weights[e]
            for ts in range(nb // P):
                col = t0 // P + ts
                yt = ypool.tile([P, D], bf, tag="yt", name=f"yt_{t0}_{ts}")
                for ds in range(2):
                    py = ps2.tile([P, 512], f32, tag="py",
                                  name=f"py_{t0}_{ts}_{ds}")
                    for fc in range(FC):
                        nc.tensor.matmul(py[:],
                                         lhsT=ht[:, fc, ts * P:(ts + 1) * P],
                                         rhs=w2t[:, fc, ds * 512:(ds + 1) * 512],
                                         start=(fc == 0), stop=(fc == FC - 1))
                    nc.vector.tensor_scalar_mul(yt[:, ds * 512:(ds + 1) * 512],
                                                py[:], wgt[:, col:col + 1])
                nc.sync.dma_start(YP[t0 + ts * P: t0 + (ts + 1) * P, :], yt[:])

        # one-block software skew: stage1(b+1) is emitted before stage2(b)
        # so the PE never waits on ACT/DVE finishing hT of the current block.
        prev = None
        for blk in blocks:
            e = blk[0]
            if e not in weights:
                weights[e] = load_weights(e)
            ht_cur = stage1(blk)
            if prev is not None:
                stage2(*prev)
            prev = (blk, ht_cur)
        if prev is not None:
            stage2(*prev)

    nc.finalize()
    return nc


def kernel(x, gate_w, W1, W3, W2):
    global LAST_RESULTS, LAST_NC
    from concourse.bass_utils import run_bass_kernel_spmd

    x = np.asarray(x, dtype=np.float32)
    gate_w = np.asarray(gate_w, dtype=np.float32)
    W1 = np.asarray(W1, dtype=np.float32)
    W3 = np.asarray(W3, dtype=np.float32)
    W2 = np.asarray(W2, dtype=np.float32)

    xt = x.reshape(T, D)

    # ---- host routing (replicates reference on jax-CPU) ----
    _, top_w, top_idx = _route_host(xt, gate_w)
    pair_tok, pair_w, seg, blocks, T2 = _build_schedule(top_idx, top_w)

    # ---- device input staging ----
    g = xt[pair_tok]                                      # [T2, D] f32
    XGh = np.ascontiguousarray(
        g.T.reshape(DC, P, T2).transpose(1, 0, 2)).astype(BF16)
    WGh = np.ascontiguousarray(pair_w.reshape(T2 // P, P).T)

    in_maps = []
    for c in range(NCORES):
        w1s = np.ascontiguousarray(
            W1[:, :, c * FSL:(c + 1) * FSL].reshape(E, DC, P, FSL)
            .transpose(0, 2, 1, 3)).astype(BF16)
        w3s = np.ascontiguousarray(
            W3[:, :, c * FSL:(c + 1) * FSL].reshape(E, DC, P, FSL)
            .transpose(0, 2, 1, 3)).astype(BF16)
        w2s = np.ascontiguousarray(
            W2[:, c * FSL:(c + 1) * FSL, :].reshape(E, FC, P, D)
            .transpose(0, 2, 1, 3)).astype(BF16)
        xrc = np.ascontiguousarray(
            xt[c * TR:(c + 1) * TR].T.reshape(DC, P, TR).transpose(1, 0, 2))
        gwt = np.ascontiguousarray(
            gate_w.T.reshape(DC, P, E).transpose(1, 0, 2))
        in_maps.append({
            "xg": XGh, "wg": WGh.astype(np.float32),
            "w1s": w1s, "w3s": w3s, "w2s": w2s,
            "xr": xrc.astype(np.float32), "gwt": gwt.astype(np.float32),
        })

    nc = _build_program(blocks, T2)
    LAST_NC = nc
    res = run_bass_kernel_spmd(nc, in_maps, core_ids=list(range(NCORES)),
                               trace=False)
    LAST_RESULTS = res

    # ---- host combine: sum F-slice partials, scatter-add per expert ----
    ysum = np.zeros((T2, D), dtype=np.float32)
    for c in range(NCORES):
        ysum += res.results[c]["yp"].astype(np.float32)
    out = np.zeros((T, D), dtype=np.float32)
    for e, off, creal in seg:
        rows, cols = np.nonzero(top_idx == e)
        out[rows] += ysum[off:off + creal]
    logits = np.concatenate([res.results[c]["lg"] for c in range(NCORES)],
                            axis=0)
    return out.reshape(B, S, D), logits
